# revision 33
# baseline (speedup 1.0000x reference)
"""Trainium2 Bass kernel for an attention layer.

Computes, for each batch element b:
    q      = x @ W                  [T, D]
    scores = q @ x^T                [T, T]
    out    = softmax(scores) @ x    [T, D]

with B=8, T=4096, D=64, f32 in/out. Sharding: data-parallel over batch,
one batch element per NeuronCore (8 cores), W replicated. No collectives.

Per-core algorithm (flash-style, scores never touch HBM), p-major row
layout: SBUF partition p holds x rows {p*32 + j, j=0..31}, so the input
DMA is one contiguous read per partition (small first chunk so compute
starts early). All row/column orders are consistently permuted
(t = c*32 + k); the strided output DMAs un-permute, hidden under the
main loop.

  - xT/qT [128, T] bf16 via PE transposes + W^T stationary matmul,
    duplicated onto both PE row-halves (row-group alternation lets the
    LDWEIGHTS of consecutive score matmuls overlap). Emitted as thunks
    drip-fed between early main-loop steps.
  - x_aug [T, 128] bf16: x | ones column (row sums come free in the AV
    matmul) | zero padding to 128 columns so LDWEIGHTS is FWL-eligible.
  - per 512-col panel of t, 16 groups of 2 s-blocks, software-pipelined
    flat across panels (scores run `look` groups ahead; a panel's
    epilogue is emitted two groups into the next panel):
      scores (PE, alternating row halves) -> PSUM f32
      exp: ScalarE Exp for 9 groups, DVE Schraudolph-bf16 for 7
      o_augT[128, 512] += x_aug^T @ exp (PE accumulate)
    epilogue: PE-transpose, divide by the sums column, DMA out.
"""

import numpy as np

B, T, D = 8, 4096, 64
P = 128                 # SBUF/PSUM partitions
NBLK = T // P           # 32 s-blocks
PW = 512                # panel width (t columns per panel)
NPANEL = T // PW        # 8 panels
DA = D + 1              # augmented with ones column


# Schraudolph exp in bf16 bits: exp(x) ~= bitcast_bf16(int16(A*x + BIAS))
EXP_A = 128.0 / np.log(2.0)
EXP_BIAS = 127.0 * 128.0 - 5.59


def build_bass(stage="full", reps=1, loop=1, rt=True, av_split=False,
               dve_n=7, ablate=(), dma="sync4", grp=2, fwl_pad=True,
               look=1, pair=False, sc_bf=False, drip=True,
               dve_late=False, o1=False, hints=True, exb=3):
    import contextlib
    import concourse.bacc as bacc
    import concourse.mybir as mybir
    import concourse.tile as tile
    from concourse.masks import make_identity

    f32 = mybir.dt.float32
    bf16 = mybir.dt.bfloat16
    i16 = mybir.dt.int16
    EXP = mybir.ActivationFunctionType.Exp
    MULT, ADD = mybir.AluOpType.mult, mybir.AluOpType.add
    ablate = set(ablate)
    GSZ = grp
    NGRP = NBLK // GSZ
    XW = P if fwl_pad else DA   # x_aug column count (pad for FWL)

    # spread the DVE-exp groups evenly among the NGRP; dve_late=True
    # packs them into the tail half-pattern (odd groups from the top) so
    # the DVE's prologue copies don't collide with early exps
    dve_set = set()
    if dve_n > 0 and dve_late:
        dve_set = {NGRP - 1 - 2 * i for i in range(min(dve_n, NGRP // 2))}
        i = 0
        while len(dve_set) < dve_n:
            if i not in dve_set:
                dve_set.add(i)
            i += 1
    elif dve_n > 0:
        for i in range(dve_n):
            dve_set.add(int(round((i + 0.5) * NGRP / dve_n)) % NGRP)
        # rounding collisions: fill greedily
        i = 0
        while len(dve_set) < dve_n:
            if i not in dve_set:
                dve_set.add(i)
            i += 1

    nc = bacc.Bacc("TRN2", target_bir_lowering=False, debug=False, num_devices=B)

    x_ext = nc.dram_tensor("x", [T, D], f32, kind="ExternalInput")
    w_ext = nc.dram_tensor("W", [D, D], f32, kind="ExternalInput")
    out_ext = nc.dram_tensor("out", [T, D], f32, kind="ExternalOutput")

    # partition p <- rows p*NBLK + j (contiguous per partition)
    x_view = x_ext.ap().rearrange("(p j) d -> p j d", j=NBLK)
    # epilogue chunk m holds out rows {c*NBLK + m : c in [0,128)}
    out_view = out_ext.ap().rearrange("(c m) d -> m c d", m=NBLK)

    xparts = P if rt else D

    with tile.TileContext(nc) as tc:
        with (
            tc.tile_pool(name="const", bufs=1) as const,
            tc.tile_pool(name="sb", bufs=1) as sb,
            tc.tile_pool(name="aux_ps", bufs=1 if o1 else 2,
                         space="PSUM") as aux_ps,
            tc.tile_pool(name="sc_ps", bufs=look + 1, space="PSUM") as sc_ps,
            tc.tile_pool(name="o_ps", bufs=1 if (pair or o1) else 2,
                         space="PSUM") as o_ps,
            tc.tile_pool(name="exps", bufs=look + exb) as exps,
            tc.tile_pool(name="osb", bufs=2) as osb,
            tc.tile_pool(name="small", bufs=4) as small,
        ):
          HINT = (tuple(mybir.EngineType(e) for e in
                        ("PE", "DVE", "Activation", "SP", "Pool"))
                  if hints else ())
          loop_cm = (tc.For_i(0, loop, 1, hint_engines=HINT) if loop > 1
                     else contextlib.nullcontext())
          with loop_cm:
           for _rep in range(reps):
            pstage = int(stage[1]) if (
                len(stage) == 2 and stage[0] == "p") else 99
            out_dbg = out_ext.ap().rearrange("(a b) d -> a (b d)", a=D)

            ident = const.tile([P, P], f32)
            make_identity(nc, ident[:])

            # chunked contiguous input DMA first (small first chunk so
            # the first transposes start early); W afterwards (not
            # needed until the first q matmul)
            x_sb = sb.tile([P, NBLK, D], f32)
            bounds = ([0, 4, 12, 20, 32] if dma == "sync4"
                      else [0, NBLK])
            eng = nc.gpsimd if dma.startswith("gps") else nc.sync
            if dma not in ("sync4", "sync1"):
                n_chunk = int(dma[-1])
                CB = NBLK // n_chunk
                bounds = [c * CB for c in range(n_chunk)] + [NBLK]
            w_sb = const.tile([D, D], f32)
            w_bf = const.tile([D, D], bf16)
            for c in range(len(bounds) - 1):
                sl = slice(bounds[c], bounds[c + 1])
                eng.dma_start(out=x_sb[:, sl, :], in_=x_view[:, sl, :])
                if c == 0:
                    # W rides second in the queue: tiny, and needed by
                    # the first q matmul at ~3us
                    nc.sync.dma_start(out=w_sb[:], in_=w_ext.ap())
                    nc.gpsimd.tensor_copy(w_bf[:], w_sb[:])
            if pstage == 0:
                nc.gpsimd.dma_start(out=out_dbg[:, 0:NBLK * D],
                                    in_=x_sb[0:D, :, :])

            # x_aug: [P, NBLK, XW] bf16, ones in column D (zeros beyond
            # when padded to 128 columns for FWL-eligible LDWEIGHTS)
            x_aug = sb.tile([P, NBLK, XW], bf16)
            if pstage >= 1:
                # on GPSIMD (idle in the prologue) so the DVE can feed
                # the xT/qT copies and early exps instead; contiguous
                # full-tile memset (strided memsets are slow on the Q7)
                if fwl_pad:
                    nc.gpsimd.memset(x_aug[:], 0.0)
                    nc.gpsimd.memset(x_aug[:, :, D:DA], 1.0)
                else:
                    nc.gpsimd.memset(x_aug[:], 1.0)
                for c in range(len(bounds) - 1):
                    sl = slice(bounds[c], bounds[c + 1])
                    nc.gpsimd.tensor_copy(x_aug[:, sl, 0:D], x_sb[:, sl, :])
            if pstage == 1:
                nc.gpsimd.dma_start(
                    out=out_dbg[:, 0:NBLK * D // 2],
                    in_=x_aug[0:D, :, 0:D].bitcast(f32))

            # xT / qT [xparts, T] bf16 via PE transposes and the W^T
            # stationary matmul; with dup copies onto the second row half
            # (ScalarE) when row-tiling. Emitted as interleaved thunks
            # t0,q0,t1,q1,... — the first few run before the main loop,
            # the rest are drip-fed between main-loop steps so the PE
            # starts scoring panel 0 early.
            xT = sb.tile([xparts, T], bf16)
            qT = sb.tile([xparts, T], bf16)

            def emit_trans(r):
                tp = aux_ps.tile([D, 4 * P], f32, tag="aux", name="tp")
                for j in range(4):
                    blk = 4 * r + j
                    nc.tensor.transpose(
                        tp[:, j * P:(j + 1) * P], x_sb[:, blk, :],
                        ident[:]
                    )
                sl = slice(r * 4 * P, (r + 1) * 4 * P)
                nc.vector.tensor_copy(xT[0:D, sl], tp[:])
                if rt:
                    # dup from SBUF, not PSUM: frees the tp buffer after
                    # the DVE copy so the next transposes start sooner
                    nc.scalar.copy(xT[D:2 * D, sl], xT[0:D, sl])

            def emit_q(j):
                qp = aux_ps.tile([D, PW], f32, tag="aux", name="qp")
                nc.tensor.matmul(
                    qp[:], w_bf[:], xT[0:D, j * PW:(j + 1) * PW],
                    start=True, stop=True,
                )
                sl = slice(j * PW, (j + 1) * PW)
                nc.vector.tensor_copy(qT[0:D, sl], qp[:])
                if rt:
                    nc.scalar.copy(qT[D:2 * D, sl], qT[0:D, sl])

            pro = []
            if pstage >= 2:
                for r in range(NBLK // 4):
                    pro.append(lambda r=r: emit_trans(r))
                    if pstage >= 3:
                        pro.append(lambda r=r: emit_q(r))
            if drip and (stage == "full" or stage == "panel1"):
                # keep t0,q0,t1,q1 up front; drip the rest into the loop
                for th in pro[:4]:
                    th()
                pro = pro[4:]
            else:
                for th in pro:
                    th()
                pro = []
            if pstage == 2:
                nc.gpsimd.dma_start(out=out_dbg[:, 0:T // 2],
                                    in_=xT[0:D, :].bitcast(f32))
            if pstage == 3:
                nc.gpsimd.dma_start(out=out_dbg[:, 0:T // 2],
                                    in_=qT[0:D, :].bitcast(f32))

            if stage == "prologue":
                # debug: dump qT into out rows (reinterpret out as [64, 4096])
                nc.gpsimd.dma_start(out=out_dbg, in_=qT[0:D, :])

            panels = [] if (stage == "prologue" or pstage <= 3) else (
                [0] if stage == "panel1" else list(range(NPANEL)))

            def emit_sc(sc, pnl, i):
                if "noscores" in ablate:
                    return
                for h in range(GSZ):
                    k = GSZ * i + h
                    base = D * (k % 2) if rt else 0
                    nc.tensor.matmul(
                        sc[:, h * PW:(h + 1) * PW],
                        xT[base:base + D, k * P:(k + 1) * P],
                        qT[base:base + D, pnl * PW:(pnl + 1) * PW],
                        start=True, stop=True,
                    )

            def emit_exp(ex, sc, i):
                if "noact" in ablate:
                    return
                if i in dve_set:
                    nc.vector.tensor_scalar(
                        ex[:].bitcast(i16), sc[:], EXP_A, EXP_BIAS,
                        MULT, ADD,
                    )
                else:
                    nc.scalar.activation(ex[:], sc[:], EXP)

            def emit_av(op, ex, i):
                if "noav" in ablate:
                    return
                for h in range(GSZ):
                    k = GSZ * i + h
                    exs = ex[:, h * PW:(h + 1) * PW]
                    nc.tensor.matmul(
                        op[:], x_aug[:, k, :], exs[:],
                        start=(k == 0), stop=(k == NBLK - 1),
                    )

            def emit_epilogue(pnl, op):
                ob = osb.tile([DA, PW], f32)
                nc.vector.tensor_copy(ob[:], op[0:DA, :])
                for j in range(PW // P):
                    tp2 = aux_ps.tile([P, DA], f32, tag="aux")
                    nc.tensor.transpose(
                        tp2[:], ob[:, j * P:(j + 1) * P], ident[0:DA, 0:DA]
                    )
                    rc = small.tile([P, 1], f32, tag="rc")
                    nc.vector.reciprocal(rc[:], tp2[:, D:DA])
                    rs = small.tile([P, D], f32, tag="rs")
                    nc.vector.tensor_scalar_mul(rs[:], tp2[:, 0:D], rc[:])
                    nc.sync.dma_start(
                        out=out_view[pnl * (PW // P) + j], in_=rs[:]
                    )

            if not pair:
                # software-pipelined main loop, flattened across panels:
                # scores run `look` groups ahead of the AV accumulations
                # (lookahead crosses panel boundaries); a panel's epilogue
                # is emitted two groups into the next panel.
                steps = [(pnl, i) for pnl in panels for i in range(NGRP)]
                mk_sc = lambda: sc_ps.tile([P, GSZ * PW], f32,
                                           tag="sc", name="sc")
                mk_ex = lambda: exps.tile([P, GSZ * PW], bf16, tag="ex",
                                          name="ex")
                if "noscores" in ablate and steps:
                    sc_shared = mk_sc()
                    nc.vector.memset(sc_shared[:], 0.0)
                    mk_sc = lambda: sc_shared
                if "noact" in ablate and steps:
                    ex_shared = mk_ex()
                    nc.vector.memset(ex_shared[:], 1.0)
                    mk_ex = lambda: ex_shared
                sc_t = {}
                for j in range(min(look + 1, len(steps))):
                    pn, ii = steps[j]
                    sc_t[j] = mk_sc()
                    emit_sc(sc_t[j], pn, ii)
                ops = {}
                pend = None
                for j, (pnl, i) in enumerate(steps):
                    if pro:
                        pro.pop(0)()
                    if i == 0:
                        ops[pnl] = o_ps.tile([XW, PW], f32, name="op")
                        if "noav" in ablate:
                            nc.vector.memset(ops[pnl][:], 0.0)
                    ex_i = mk_ex()
                    emit_exp(ex_i, sc_t.pop(j), i)
                    jn = j + look + 1
                    if jn < len(steps):
                        pn, ii = steps[jn]
                        sc_t[jn] = mk_sc()
                        emit_sc(sc_t[jn], pn, ii)
                    if pend is not None and i == (0 if o1 else 1):
                        emit_epilogue(*pend)
                        pend = None
                    emit_av(ops[pnl], ex_i, i)
                    if i == NGRP - 1:
                        pend = (pnl, ops.pop(pnl))
                if pend is not None:
                    emit_epilogue(*pend)
                    pend = None
            else:
                # pair mode: 1024-col windows (2 panels per matmul), one
                # s-block per step; sc/AV matmul and LDWEIGHTS count
                # halved. op is single-buffered: the epilogue of window w
                # is emitted right before av(w+1, 0) so the WAR edge
                # (op reuse) lands after the DVE drain copy.
                WW = 2 * PW
                wins = [] if not panels else list(range(T // WW))
                if stage == "panel1":
                    wins = [0]
                steps = len(wins) * NBLK
                mk_sc2 = lambda: sc_ps.tile([P, WW], f32, tag="sc",
                                            name="sc")
                mk_ex2 = lambda: exps.tile([P, WW], bf16, tag="ex",
                                           name="ex")

                def emit_sc2(sc, w, k):
                    if "noscores" in ablate:
                        nc.vector.memset(sc[:], 0.0)
                        return
                    base = D * (k % 2) if rt else 0
                    nc.tensor.matmul(
                        sc[:],
                        xT[base:base + D, k * P:(k + 1) * P],
                        qT[base:base + D, w * WW:(w + 1) * WW],
                        start=True, stop=True,
                    )

                def emit_exp2(ex, sc, k):
                    if "noact" in ablate:
                        nc.vector.memset(ex[:], 1.0)
                        return
                    if k in dve_set:
                        nc.vector.tensor_scalar(
                            ex[:].bitcast(i16), sc[:], EXP_A, EXP_BIAS,
                            MULT, ADD,
                        )
                    else:
                        nc.scalar.activation(ex[:], sc[:], EXP)

                def emit_av2(op, ex, k):
                    if "noav" in ablate:
                        return
                    nc.tensor.matmul(
                        op[:], x_aug[:, k, :], ex[:],
                        start=(k == 0), stop=(k == NBLK - 1),
                    )

                def emit_epilogue2(w, op):
                    ob = osb.tile([DA, WW], f32, name="ob")
                    nc.vector.tensor_copy(ob[:], op[0:DA, :])
                    for j in range(WW // P):
                        tp2 = aux_ps.tile([P, DA], f32, tag="aux",
                                          name="tp2")
                        nc.tensor.transpose(
                            tp2[:], ob[:, j * P:(j + 1) * P],
                            ident[0:DA, 0:DA]
                        )
                        rc = small.tile([P, 1], f32, tag="rc", name="rc")
                        nc.vector.reciprocal(rc[:], tp2[:, D:DA])
                        rs = small.tile([P, D], f32, tag="rs", name="rs")
                        nc.vector.tensor_scalar_mul(rs[:], tp2[:, 0:D],
                                                    rc[:])
                        nc.sync.dma_start(
                            out=out_view[w * (WW // P) + j], in_=rs[:]
                        )

                sc_t = {}
                ops = {}
                pend2 = None
                for j in range(min(look + 1, steps)):
                    w, k = wins[j // NBLK], j % NBLK
                    sc_t[j] = mk_sc2()
                    emit_sc2(sc_t[j], w, k)
                for j in range(steps):
                    w, k = wins[j // NBLK], j % NBLK
                    if k == 0:
                        ops[w] = o_ps.tile([XW, WW], f32, name="op")
                        if "noav" in ablate:
                            nc.vector.memset(ops[w][:], 0.0)
                    ex_j = mk_ex2()
                    emit_exp2(ex_j, sc_t.pop(j), k)
                    jn = j + look + 1
                    if jn < steps:
                        wn, kn = wins[jn // NBLK], jn % NBLK
                        sc_t[jn] = mk_sc2()
                        emit_sc2(sc_t[jn], wn, kn)
                    if k == 0 and pend2 is not None:
                        emit_epilogue2(*pend2)
                        pend2 = None
                    emit_av2(ops[w], ex_j, k)
                    if k == NBLK - 1:
                        pend2 = (w, ops.pop(w))
                if pend2 is not None:
                    emit_epilogue2(*pend2)

    if not nc.is_finalized():
        nc.finalize()
    return nc


CHECK_VARIANTS = {
    "full": dict(),
    "nodve": dict(dve_n=0),
    "nort": dict(rt=False, av_split=False),
    "dve4": dict(dve_n=4),
    "dve10": dict(dve_n=10),
    "dve16": dict(dve_n=16),
}


def kernel(inputs: np.ndarray, W: np.ndarray) -> np.ndarray:
    from concourse.bass_utils import run_bass_kernel_spmd

    nc = build_bass()
    x = np.ascontiguousarray(np.asarray(inputs, dtype=np.float32))
    w = np.ascontiguousarray(np.asarray(W, dtype=np.float32))
    in_maps = [{"x": x[i], "W": w} for i in range(B)]
    res = run_bass_kernel_spmd(nc, in_maps, core_ids=list(range(B)))
    out = np.stack([res.results[i]["out"] for i in range(B)], axis=0)
    return out.astype(np.float32)


if __name__ == "__main__":
    rng = np.random.default_rng(0)
    x = rng.standard_normal((B, T, D), dtype=np.float32)
    w = (rng.standard_normal((D, D)) * 0.05).astype(np.float32)
    out = kernel(inputs=x, W=w)
    print("out", out.shape, out.dtype)


# revision 36
# speedup vs baseline: 1.0695x; 1.0695x over previous
"""Trainium2 Bass kernel for an attention layer.

Computes, for each batch element b:
    q      = x @ W                  [T, D]
    scores = q @ x^T                [T, T]
    out    = softmax(scores) @ x    [T, D]

with B=8, T=4096, D=64, f32 in/out. Sharding: data-parallel over batch,
one batch element per NeuronCore (8 cores), W replicated. No collectives.

Per-core algorithm (flash-style, scores never touch HBM), p-major row
layout: SBUF partition p holds x rows {p*32 + j, j=0..31}, so the input
DMA is one contiguous read per partition (small first chunk so compute
starts early). All row/column orders are consistently permuted
(t = c*32 + k); the strided output DMAs un-permute, hidden under the
main loop.

  - xT/qT [128, T] bf16 via PE transposes + W^T stationary matmul,
    duplicated onto both PE row-halves (row-group alternation lets the
    LDWEIGHTS of consecutive score matmuls overlap). Emitted as thunks
    drip-fed between early main-loop steps.
  - x_aug [T, 128] bf16: x | ones column (row sums come free in the AV
    matmul) | zero padding to 128 columns so LDWEIGHTS is FWL-eligible.
  - per 512-col panel of t, 16 groups of 2 s-blocks, software-pipelined
    flat across panels (scores run `look` groups ahead; a panel's
    epilogue is emitted two groups into the next panel):
      scores (PE, alternating row halves) -> PSUM f32
      exp: ScalarE Exp for 9 groups, DVE Schraudolph-bf16 for 7
      o_augT[128, 512] += x_aug^T @ exp (PE accumulate)
    epilogue: PE-transpose, divide by the sums column, DMA out.
"""

import numpy as np

B, T, D = 8, 4096, 64
P = 128                 # SBUF/PSUM partitions
NBLK = T // P           # 32 s-blocks
PW = 512                # panel width (t columns per panel)
NPANEL = T // PW        # 8 panels
DA = D + 1              # augmented with ones column


# Schraudolph exp in bf16 bits: exp(x) ~= bitcast_bf16(int16(A*x + BIAS))
EXP_A = 128.0 / np.log(2.0)
EXP_BIAS = 127.0 * 128.0 - 5.59


def build_bass(stage="full", reps=1, loop=1, rt=True, av_split=False,
               dve_n=7, ablate=(), dma="sync4", grp=2, fwl_pad=True,
               look=1, pair=False, sc_bf=False, drip=True,
               dve_late=False, o1=False, hints=True, exb=3):
    import contextlib
    import concourse.bacc as bacc
    import concourse.mybir as mybir
    import concourse.tile as tile
    from concourse.masks import make_identity

    f32 = mybir.dt.float32
    bf16 = mybir.dt.bfloat16
    i16 = mybir.dt.int16
    EXP = mybir.ActivationFunctionType.Exp
    MULT, ADD = mybir.AluOpType.mult, mybir.AluOpType.add
    ablate = set(ablate)
    GSZ = grp
    NGRP = NBLK // GSZ
    XW = P if fwl_pad else DA   # x_aug column count (pad for FWL)

    # spread the DVE-exp groups evenly among the NGRP; dve_late=True
    # packs them into the tail half-pattern (odd groups from the top) so
    # the DVE's prologue copies don't collide with early exps
    dve_set = set()
    if dve_n > 0 and dve_late:
        dve_set = {NGRP - 1 - 2 * i for i in range(min(dve_n, NGRP // 2))}
        i = 0
        while len(dve_set) < dve_n:
            if i not in dve_set:
                dve_set.add(i)
            i += 1
    elif dve_n > 0:
        for i in range(dve_n):
            dve_set.add(int(round((i + 0.5) * NGRP / dve_n)) % NGRP)
        # rounding collisions: fill greedily
        i = 0
        while len(dve_set) < dve_n:
            if i not in dve_set:
                dve_set.add(i)
            i += 1

    nc = bacc.Bacc("TRN2", target_bir_lowering=False, debug=False, num_devices=B)

    x_ext = nc.dram_tensor("x", [T, D], f32, kind="ExternalInput")
    w_ext = nc.dram_tensor("W", [D, D], f32, kind="ExternalInput")
    out_ext = nc.dram_tensor("out", [T, D], f32, kind="ExternalOutput")

    # partition p <- rows p*NBLK + j (contiguous per partition)
    x_view = x_ext.ap().rearrange("(p j) d -> p j d", j=NBLK)
    # epilogue chunk m holds out rows {c*NBLK + m : c in [0,128)};
    # paired view: chunk-pair m2 covers rows c*NBLK + 2*m2 + {0,1}
    out_view = out_ext.ap().rearrange("(c m) d -> m c d", m=NBLK)
    out_pair = out_ext.ap().rearrange("(c m2 t) d -> m2 c (t d)", t=2,
                                      m2=NBLK // 2)
    out_quad = out_ext.ap().rearrange("(c m4 t) d -> m4 c (t d)", t=4,
                                      m4=NBLK // 4)

    xparts = P if rt else D

    with tile.TileContext(nc) as tc:
        with (
            tc.tile_pool(name="const", bufs=1) as const,
            tc.tile_pool(name="sb", bufs=1) as sb,
            tc.tile_pool(name="aux_ps", bufs=1 if o1 else 2,
                         space="PSUM") as aux_ps,
            tc.tile_pool(name="sc_ps", bufs=look + 1, space="PSUM") as sc_ps,
            tc.tile_pool(name="o_ps", bufs=1 if (pair or o1) else 2,
                         space="PSUM") as o_ps,
            tc.tile_pool(name="exps", bufs=look + exb) as exps,
            tc.tile_pool(name="osb", bufs=2) as osb,
            tc.tile_pool(name="small", bufs=4) as small,
        ):
          HINT = (tuple(mybir.EngineType(e) for e in
                        ("PE", "DVE", "Activation", "SP", "Pool"))
                  if hints else ())
          loop_cm = (tc.For_i(0, loop, 1, hint_engines=HINT) if loop > 1
                     else contextlib.nullcontext())
          with loop_cm:
           for _rep in range(reps):
            pstage = int(stage[1]) if (
                len(stage) == 2 and stage[0] == "p") else 99
            out_dbg = out_ext.ap().rearrange("(a b) d -> a (b d)", a=D)

            ident = const.tile([P, P], f32)
            make_identity(nc, ident[:])

            # chunked contiguous input DMA first (small first chunk so
            # the first transposes start early); W afterwards (not
            # needed until the first q matmul)
            x_sb = sb.tile([P, NBLK, D], f32)
            bounds = ([0, 4, 12, 20, 32] if dma == "sync4"
                      else [0, NBLK])
            eng = nc.gpsimd if dma.startswith("gps") else nc.sync
            if dma not in ("sync4", "sync1"):
                n_chunk = int(dma[-1])
                CB = NBLK // n_chunk
                bounds = [c * CB for c in range(n_chunk)] + [NBLK]
            w_sb = const.tile([D, D], f32)
            w_bf = const.tile([D, D], bf16)
            for c in range(len(bounds) - 1):
                sl = slice(bounds[c], bounds[c + 1])
                eng.dma_start(out=x_sb[:, sl, :], in_=x_view[:, sl, :])
                if c == 0:
                    # W rides second in the queue: tiny, and needed by
                    # the first q matmul at ~3us
                    nc.sync.dma_start(out=w_sb[:], in_=w_ext.ap())
                    nc.gpsimd.tensor_copy(w_bf[:], w_sb[:])
            if pstage == 0:
                nc.gpsimd.dma_start(out=out_dbg[:, 0:NBLK * D],
                                    in_=x_sb[0:D, :, :])

            # x_aug: [P, NBLK, XW] bf16, ones in column D (zeros beyond
            # when padded to 128 columns for FWL-eligible LDWEIGHTS)
            x_aug = sb.tile([P, NBLK, XW], bf16)
            if pstage >= 1:
                # on GPSIMD (idle in the prologue) so the DVE can feed
                # the xT/qT copies and early exps instead; contiguous
                # full-tile memset (strided memsets are slow on the Q7)
                if fwl_pad:
                    nc.gpsimd.memset(x_aug[:], 0.0)
                    nc.gpsimd.memset(x_aug[:, :, D:DA], 1.0)
                else:
                    nc.gpsimd.memset(x_aug[:], 1.0)
                for c in range(len(bounds) - 1):
                    sl = slice(bounds[c], bounds[c + 1])
                    nc.gpsimd.tensor_copy(x_aug[:, sl, 0:D], x_sb[:, sl, :])
            if pstage == 1:
                nc.gpsimd.dma_start(
                    out=out_dbg[:, 0:NBLK * D // 2],
                    in_=x_aug[0:D, :, 0:D].bitcast(f32))

            # xT / qT [xparts, T] bf16 via PE transposes and the W^T
            # stationary matmul; with dup copies onto the second row half
            # (ScalarE) when row-tiling. Emitted as interleaved thunks
            # t0,q0,t1,q1,... — the first few run before the main loop,
            # the rest are drip-fed between main-loop steps so the PE
            # starts scoring panel 0 early.
            xT = sb.tile([xparts, T], bf16)
            qT = sb.tile([xparts, T], bf16)

            def emit_trans(r):
                tp = aux_ps.tile([D, 4 * P], f32, tag="aux", name="tp")
                for j in range(4):
                    blk = 4 * r + j
                    nc.tensor.transpose(
                        tp[:, j * P:(j + 1) * P], x_sb[:, blk, :],
                        ident[:]
                    )
                sl = slice(r * 4 * P, (r + 1) * 4 * P)
                nc.vector.tensor_copy(xT[0:D, sl], tp[:])
                if rt:
                    # dup from SBUF, not PSUM: frees the tp buffer after
                    # the DVE copy so the next transposes start sooner
                    nc.scalar.copy(xT[D:2 * D, sl], xT[0:D, sl])

            def emit_q(j):
                qp = aux_ps.tile([D, PW], f32, tag="aux", name="qp")
                nc.tensor.matmul(
                    qp[:], w_bf[:], xT[0:D, j * PW:(j + 1) * PW],
                    start=True, stop=True,
                )
                sl = slice(j * PW, (j + 1) * PW)
                nc.vector.tensor_copy(qT[0:D, sl], qp[:])
                if rt:
                    nc.scalar.copy(qT[D:2 * D, sl], qT[0:D, sl])

            pro = []
            if pstage >= 2:
                for r in range(NBLK // 4):
                    pro.append(lambda r=r: emit_trans(r))
                    if pstage >= 3:
                        pro.append(lambda r=r: emit_q(r))
            if drip and (stage == "full" or stage == "panel1"):
                # keep t0,q0,t1,q1 up front; drip the rest into the loop
                for th in pro[:4]:
                    th()
                pro = pro[4:]
            else:
                for th in pro:
                    th()
                pro = []
            if pstage == 2:
                nc.gpsimd.dma_start(out=out_dbg[:, 0:T // 2],
                                    in_=xT[0:D, :].bitcast(f32))
            if pstage == 3:
                nc.gpsimd.dma_start(out=out_dbg[:, 0:T // 2],
                                    in_=qT[0:D, :].bitcast(f32))

            if stage == "prologue":
                # debug: dump qT into out rows (reinterpret out as [64, 4096])
                nc.gpsimd.dma_start(out=out_dbg, in_=qT[0:D, :])

            panels = [] if (stage == "prologue" or pstage <= 3) else (
                [0] if stage == "panel1" else list(range(NPANEL)))

            def emit_sc(sc, pnl, i):
                if "noscores" in ablate:
                    return
                for h in range(GSZ):
                    k = GSZ * i + h
                    base = D * (k % 2) if rt else 0
                    nc.tensor.matmul(
                        sc[:, h * PW:(h + 1) * PW],
                        xT[base:base + D, k * P:(k + 1) * P],
                        qT[base:base + D, pnl * PW:(pnl + 1) * PW],
                        start=True, stop=True,
                    )

            def emit_exp(ex, sc, i):
                if "noact" in ablate:
                    return
                if i in dve_set:
                    nc.vector.tensor_scalar(
                        ex[:].bitcast(i16), sc[:], EXP_A, EXP_BIAS,
                        MULT, ADD,
                    )
                else:
                    nc.scalar.activation(ex[:], sc[:], EXP)

            def emit_av(op, ex, i):
                if "noav" in ablate:
                    return
                for h in range(GSZ):
                    k = GSZ * i + h
                    exs = ex[:, h * PW:(h + 1) * PW]
                    nc.tensor.matmul(
                        op[:], x_aug[:, k, :], exs[:],
                        start=(k == 0), stop=(k == NBLK - 1),
                    )

            def emit_epilogue(pnl, op):
                ob = osb.tile([DA, PW], f32)
                nc.vector.tensor_copy(ob[:], op[0:DA, :])
                # all four 128-row chunks of the panel in ONE DMA:
                # chunks m..m+3 are interleaved rows c*32+m+t, so the
                # quad store is 1KB-contiguous per c in HBM
                rs4 = small.tile([P, 4, D], f32, tag="rs", name="rs4")
                for j in range(PW // P):
                    tp2 = aux_ps.tile([P, DA], f32, tag="aux",
                                      name="tp2")
                    nc.tensor.transpose(
                        tp2[:], ob[:, j * P:(j + 1) * P],
                        ident[0:DA, 0:DA]
                    )
                    rc = small.tile([P, 1], f32, tag="rc", name="rc")
                    nc.vector.reciprocal(rc[:], tp2[:, D:DA])
                    nc.vector.tensor_scalar_mul(rs4[:, j, :],
                                                tp2[:, 0:D], rc[:])
                nc.sync.dma_start(out=out_quad[pnl], in_=rs4[:])

            if not pair:
                # software-pipelined main loop, flattened across panels:
                # scores run `look` groups ahead of the AV accumulations
                # (lookahead crosses panel boundaries); a panel's epilogue
                # is emitted two groups into the next panel.
                steps = [(pnl, i) for pnl in panels for i in range(NGRP)]
                mk_sc = lambda: sc_ps.tile([P, GSZ * PW], f32,
                                           tag="sc", name="sc")
                mk_ex = lambda: exps.tile([P, GSZ * PW], bf16, tag="ex",
                                          name="ex")
                if "noscores" in ablate and steps:
                    sc_shared = mk_sc()
                    nc.vector.memset(sc_shared[:], 0.0)
                    mk_sc = lambda: sc_shared
                if "noact" in ablate and steps:
                    ex_shared = mk_ex()
                    nc.vector.memset(ex_shared[:], 1.0)
                    mk_ex = lambda: ex_shared
                sc_t = {}
                for j in range(min(look + 1, len(steps))):
                    pn, ii = steps[j]
                    sc_t[j] = mk_sc()
                    emit_sc(sc_t[j], pn, ii)
                ops = {}
                pend = None
                for j, (pnl, i) in enumerate(steps):
                    if pro:
                        pro.pop(0)()
                    if i == 0:
                        ops[pnl] = o_ps.tile([XW, PW], f32, name="op")
                        if "noav" in ablate:
                            nc.vector.memset(ops[pnl][:], 0.0)
                    ex_i = mk_ex()
                    emit_exp(ex_i, sc_t.pop(j), i)
                    jn = j + look + 1
                    if jn < len(steps):
                        pn, ii = steps[jn]
                        sc_t[jn] = mk_sc()
                        emit_sc(sc_t[jn], pn, ii)
                    if pend is not None and i == (0 if o1 else 1):
                        emit_epilogue(*pend)
                        pend = None
                    emit_av(ops[pnl], ex_i, i)
                    if i == NGRP - 1:
                        pend = (pnl, ops.pop(pnl))
                if pend is not None:
                    emit_epilogue(*pend)
                    pend = None
            else:
                # pair mode: 1024-col windows (2 panels per matmul), one
                # s-block per step; sc/AV matmul and LDWEIGHTS count
                # halved. op is single-buffered: the epilogue of window w
                # is emitted right before av(w+1, 0) so the WAR edge
                # (op reuse) lands after the DVE drain copy.
                WW = 2 * PW
                wins = [] if not panels else list(range(T // WW))
                if stage == "panel1":
                    wins = [0]
                steps = len(wins) * NBLK
                mk_sc2 = lambda: sc_ps.tile([P, WW], f32, tag="sc",
                                            name="sc")
                mk_ex2 = lambda: exps.tile([P, WW], bf16, tag="ex",
                                           name="ex")

                def emit_sc2(sc, w, k):
                    if "noscores" in ablate:
                        nc.vector.memset(sc[:], 0.0)
                        return
                    base = D * (k % 2) if rt else 0
                    nc.tensor.matmul(
                        sc[:],
                        xT[base:base + D, k * P:(k + 1) * P],
                        qT[base:base + D, w * WW:(w + 1) * WW],
                        start=True, stop=True,
                    )

                def emit_exp2(ex, sc, k):
                    if "noact" in ablate:
                        nc.vector.memset(ex[:], 1.0)
                        return
                    if k in dve_set:
                        nc.vector.tensor_scalar(
                            ex[:].bitcast(i16), sc[:], EXP_A, EXP_BIAS,
                            MULT, ADD,
                        )
                    else:
                        nc.scalar.activation(ex[:], sc[:], EXP)

                def emit_av2(op, ex, k):
                    if "noav" in ablate:
                        return
                    nc.tensor.matmul(
                        op[:], x_aug[:, k, :], ex[:],
                        start=(k == 0), stop=(k == NBLK - 1),
                    )

                def emit_epilogue2(w, op):
                    ob = osb.tile([DA, WW], f32, name="ob")
                    nc.vector.tensor_copy(ob[:], op[0:DA, :])
                    for j in range(WW // P):
                        tp2 = aux_ps.tile([P, DA], f32, tag="aux",
                                          name="tp2")
                        nc.tensor.transpose(
                            tp2[:], ob[:, j * P:(j + 1) * P],
                            ident[0:DA, 0:DA]
                        )
                        rc = small.tile([P, 1], f32, tag="rc", name="rc")
                        nc.vector.reciprocal(rc[:], tp2[:, D:DA])
                        rs = small.tile([P, D], f32, tag="rs", name="rs")
                        nc.vector.tensor_scalar_mul(rs[:], tp2[:, 0:D],
                                                    rc[:])
                        nc.sync.dma_start(
                            out=out_view[w * (WW // P) + j], in_=rs[:]
                        )

                sc_t = {}
                ops = {}
                pend2 = None
                for j in range(min(look + 1, steps)):
                    w, k = wins[j // NBLK], j % NBLK
                    sc_t[j] = mk_sc2()
                    emit_sc2(sc_t[j], w, k)
                for j in range(steps):
                    w, k = wins[j // NBLK], j % NBLK
                    if k == 0:
                        ops[w] = o_ps.tile([XW, WW], f32, name="op")
                        if "noav" in ablate:
                            nc.vector.memset(ops[w][:], 0.0)
                    ex_j = mk_ex2()
                    emit_exp2(ex_j, sc_t.pop(j), k)
                    jn = j + look + 1
                    if jn < steps:
                        wn, kn = wins[jn // NBLK], jn % NBLK
                        sc_t[jn] = mk_sc2()
                        emit_sc2(sc_t[jn], wn, kn)
                    if k == 0 and pend2 is not None:
                        emit_epilogue2(*pend2)
                        pend2 = None
                    emit_av2(ops[w], ex_j, k)
                    if k == NBLK - 1:
                        pend2 = (w, ops.pop(w))
                if pend2 is not None:
                    emit_epilogue2(*pend2)

    if not nc.is_finalized():
        nc.finalize()
    return nc


CHECK_VARIANTS = {
    "full": dict(),
    "nodve": dict(dve_n=0),
    "nort": dict(rt=False, av_split=False),
    "dve4": dict(dve_n=4),
    "dve10": dict(dve_n=10),
    "dve16": dict(dve_n=16),
}


def kernel(inputs: np.ndarray, W: np.ndarray) -> np.ndarray:
    from concourse.bass_utils import run_bass_kernel_spmd

    nc = build_bass()
    x = np.ascontiguousarray(np.asarray(inputs, dtype=np.float32))
    w = np.ascontiguousarray(np.asarray(W, dtype=np.float32))
    in_maps = [{"x": x[i], "W": w} for i in range(B)]
    res = run_bass_kernel_spmd(nc, in_maps, core_ids=list(range(B)))
    out = np.stack([res.results[i]["out"] for i in range(B)], axis=0)
    return out.astype(np.float32)


if __name__ == "__main__":
    rng = np.random.default_rng(0)
    x = rng.standard_normal((B, T, D), dtype=np.float32)
    w = (rng.standard_normal((D, D)) * 0.05).astype(np.float32)
    out = kernel(inputs=x, W=w)
    print("out", out.shape, out.dtype)
